# revision 31
# baseline (speedup 1.0000x reference)
"""2-layer GAT on 8 Trainium2 NeuronCores.

Strategy: dst-shard nodes across cores (6250 each, padded to 6272). Per layer:
each core computes node features h = x @ [W | W@att_src | W@att_dst] for its
shard, AllGathers the packed per-node rows into a replicated DRAM table, then
processes its own dst nodes in CSR tiles (128 nodes on partitions, slots along
free dim): dma_gather pulls [h | alpha_src] rows for every incoming edge,
softmax and the alpha-weighted payload sum are dense strided vector ops.
int16 gather indices limit one gather to 32768 table rows, so each tile does
two gathers (table halves lo/hi) and sums the partial numerators/denominators.
"""

import numpy as np

N = 50000
E = 800000
R = 8
NPC = N // R  # 6250 owned nodes per core
TPC = 49  # tiles of 128 nodes
NPAD = TPC * 128  # 6272 rows per shard
HALF = 4 * NPAD  # 25088 table rows per half
IN_CH = 128
HIDDEN = 32
HEADS = 4
OUT_CH = 64
NEG_SLOPE = 0.2
EL1 = 192  # L1 table row: [h1(128) | as1(4) | pad] f32, 768B
EL2 = 128  # L2 table row: [h2(64) | as2(1) | pad] f32, 512B
SENT = 6250  # sentinel row (first pad row of core 0 / core 4) in each half
CAP = 34  # supertile packing: J*(D_lo+D_hi) <= CAP


# ---------------------------------------------------------------- host planner
def _build_plan(edge_index):
    src = np.concatenate([edge_index[0], np.arange(N, dtype=np.int64)]).astype(np.int64)
    dst = np.concatenate([edge_index[1], np.arange(N, dtype=np.int64)]).astype(np.int64)
    lo = src < (N // 2)  # src owned by cores 0-3 -> table half 0

    # degree per (dst, half)
    d_lo = np.bincount(dst[lo], minlength=N)
    d_hi = np.bincount(dst[~lo], minlength=N)

    # per-core permutation: lex sort desc by (d_lo, d_hi); pads (deg 0) at end
    perms = []  # perms[c] = array of orig node ids, len NPC, permuted order
    pos = np.empty(N, dtype=np.int64)  # node id -> permuted global row
    for c in range(R):
        ids = np.arange(c * NPC, (c + 1) * NPC)
        order = np.lexsort((-d_hi[ids], -d_lo[ids]))
        p = ids[order]
        perms.append(p)
        pos[p] = c * NPAD + np.arange(NPC)

    # shared per-tile max degrees (padded rows have degree 0)
    dlo_t = np.zeros(TPC, dtype=np.int64)
    dhi_t = np.zeros(TPC, dtype=np.int64)
    for c in range(R):
        dl = d_lo[perms[c]]
        dh = d_hi[perms[c]]
        dl = np.concatenate([dl, np.zeros(NPAD - NPC, dtype=np.int64)])
        dh = np.concatenate([dh, np.zeros(NPAD - NPC, dtype=np.int64)])
        dlo_t = np.maximum(dlo_t, dl.reshape(TPC, 128).max(axis=1))
        dhi_t = np.maximum(dhi_t, dh.reshape(TPC, 128).max(axis=1))
    dlo_t = np.maximum(dlo_t, 1)
    dhi_t = np.maximum(dhi_t, 1)

    # supertiles: group J consecutive tiles, J in {4,2,1}
    supers = []  # (t0, J, Dl, Dh)
    t = 0
    while t < TPC:
        for J in (4, 2, 1):
            if t + J <= TPC:
                Dl = int(dlo_t[t : t + J].max())
                Dh = int(dhi_t[t : t + J].max())
                if J * (Dl + Dh) <= CAP or J == 1:
                    break
        supers.append((t, J, Dl, Dh))
        t += J

    slots = sum(128 * J * (Dl + Dh) for (_, J, Dl, Dh) in supers)
    real = E + N
    plan = {
        "supers": supers,
        "perms": perms,
        "pos": pos,
        "inflation": slots * R / real,
    }

    # per-core slot tables: for owned dst nodes, list of src-pos per half
    # cumcount of edges within (dst, half)
    gidx_cores = []
    W_total = sum(8 * 128 * J * (Dl + Dh) // 128 for (_, J, Dl, Dh) in supers)
    # columns of wrapped idx per gather = NI/16 = 128*J*D/16 = 8*J*D
    for c in range(R):
        own = (dst >= c * NPC) & (dst < (c + 1) * NPC)
        s_own = src[own]
        d_own = dst[own]
        half = (~(s_own < (N // 2))).astype(np.int64)  # 0 lo, 1 hi
        lpos = pos[d_own] - c * NPAD  # local permuted pos of dst, [0, NPC)
        key = lpos * 2 + half
        order = np.argsort(key, kind="stable")
        key_s = key[order]
        sp = pos[s_own][order]  # permuted global row of src
        # rank within group
        first = np.searchsorted(key_s, key_s)  # index of first occurrence
        rank = np.arange(len(key_s)) - first

        # slot arrays per tile: build dense [NPAD, Dl/Dh] with sentinel fill
        cols = []
        for t0, J, Dl, Dh in supers:
            n0 = t0 * 128
            n1 = (t0 + J) * 128
            for h, D, base, sent in ((0, Dl, 0, SENT), (1, Dh, HALF, HALF + SENT)):
                tab = np.full((n1 - n0, D), sent - base, dtype=np.int64)
                sel = (key_s % 2 == h) & (key_s // 2 >= n0) & (key_s // 2 < n1)
                rr = rank[sel]
                keep = rr < D
                tab[key_s[sel][keep] // 2 - n0, rr[keep]] = sp[sel][keep] - base
                assert keep.all(), "rank exceeded tile max degree"
                # flat[i], i = c*128 + p, c = k*J + j, node = j*128 + p
                # tab is [S, D] with S = J*128 node-pos-major
                S = n1 - n0
                flat = np.empty(S * D, dtype=np.int16)
                nodes = np.arange(S)
                j = nodes // 128
                p = nodes % 128
                for k in range(D):
                    flat[(k * J + j) * 128 + p] = tab[nodes, k]
                wrapped = flat.reshape(-1, 16)  # [NI/16, 16], flat[s*16 + q]
                w = np.empty((128, S * D // 16), dtype=np.int16)
                for q in range(8):
                    w[q * 16 : (q + 1) * 16, :] = wrapped.T
                cols.append(w)
        gidx_cores.append(np.concatenate(cols, axis=1))
    plan["gidx"] = gidx_cores
    plan["W"] = gidx_cores[0].shape[1]
    assert all(g.shape[1] == plan["W"] for g in gidx_cores)
    return plan


# ---------------------------------------------------------------- bass kernel
def _build_bass(plan, phases="ABC"):
    import concourse.bacc as bacc
    import concourse.bass as bass
    import concourse.mybir as mybir
    import concourse.tile as tile
    from concourse.masks import make_identity

    f32 = mybir.dt.float32
    i16 = mybir.dt.int16
    AX = mybir.AxisListType.X
    OP = mybir.AluOpType
    AF = mybir.ActivationFunctionType

    supers = plan["supers"]
    W = plan["W"]

    nc = bacc.Bacc(
        "TRN2",
        target_bir_lowering=False,
        debug=False,
        num_devices=R,
        num_swdge_queues=4,
        dynamic_dma_scratch_size=32768,
    )
    xT_in = nc.dram_tensor("xT", [128, NPAD], f32, kind="ExternalInput")
    gidx_in = nc.dram_tensor("gidx", [128, W], i16, kind="ExternalInput")
    wcat1_in = nc.dram_tensor("wcat1", [128, 136], f32, kind="ExternalInput")
    wcat2_in = nc.dram_tensor("wcat2", [128, 66], f32, kind="ExternalInput")
    b1_in = nc.dram_tensor("b1c", [128, 1], f32, kind="ExternalInput")
    b2_in = nc.dram_tensor("b2r", [1, 64], f32, kind="ExternalInput")
    out_d = nc.dram_tensor("out", [NPAD, 64], f32, kind="ExternalOutput")

    with tile.TileContext(nc) as tc:
        with (
            tc.tile_pool(name="const", bufs=1) as cp,
            tc.tile_pool(name="work", bufs=3) as wp,
            tc.tile_pool(name="gath", bufs=4) as gp,
            tc.tile_pool(name="psum", bufs=2, space="PSUM") as pp,
            tc.tile_pool(name="dram", bufs=1, space="DRAM") as dp,
        ):
            shard1 = dp.tile([NPAD, EL1], f32)
            table1 = dp.tile([R * NPAD, EL1], f32, addr_space="Shared")
            shard2 = dp.tile([NPAD, EL2], f32)
            table2 = dp.tile([R * NPAD, EL2], f32, addr_space="Shared")

            wcat1 = cp.tile([128, 136], f32)
            nc.sync.dma_start(out=wcat1[:], in_=wcat1_in[:])
            wcat2 = cp.tile([128, 66], f32)
            nc.sync.dma_start(out=wcat2[:], in_=wcat2_in[:])
            b1c = cp.tile([128, 1], f32)
            nc.sync.dma_start(out=b1c[:], in_=b1_in[:])
            b2p = cp.tile([1, 64], f32)
            nc.sync.dma_start(out=b2p[:1, :], in_=b2_in[:])
            b2b = cp.tile([128, 64], f32)
            nc.gpsimd.partition_broadcast(b2b[:], b2p[:1, :])
            ident = cp.tile([128, 128], f32)
            make_identity(nc, ident[:])
            idxall = cp.tile([128, W], i16)
            nc.sync.dma_start(out=idxall[:], in_=gidx_in[:])
            adbuf1 = cp.tile([128, 4 * TPC], f32)
            adbuf2 = cp.tile([128, TPC], f32)
            neg = cp.tile([128, 4], f32)
            nc.gpsimd.memset(neg[:], -1e30)
            zpad = cp.tile([128, 64], f32)
            nc.gpsimd.memset(zpad[:], 0.0)

            # ---------------- phase A: h1ext = x @ [W1|Ws1|Wd1] per owned tile
            for t in range(TPC):
                xt = wp.tile([128, 128], f32, tag="xt")
                nc.sync.dma_start(out=xt[:], in_=xT_in[:, t * 128 : (t + 1) * 128])
                psA = pp.tile([128, 136], f32, tag="psA")
                nc.tensor.matmul(psA[:], lhsT=xt[:], rhs=wcat1[:], start=True, stop=True)
                hext = wp.tile([128, 136], f32, tag="hext")
                nc.scalar.copy(out=hext[:], in_=psA[:])
                nc.vector.tensor_copy(
                    out=adbuf1[:, 4 * t : 4 * t + 4], in_=hext[:, 132:136]
                )
                nc.sync.dma_start(
                    out=shard1[t * 128 : (t + 1) * 128, 0:132], in_=hext[:, 0:132]
                )
                nc.sync.dma_start(
                    out=shard1[t * 128 : (t + 1) * 128, 132:EL1],
                    in_=zpad[:, 0 : EL1 - 132],
                )
            # pad rows: alpha_src = -1e30 so padded slots vanish in the softmax
            nc.sync.dma_start(
                out=shard1[NPC:NPAD, 128:132], in_=neg[0 : NPAD - NPC, 0:4]
            )

            nc.gpsimd.collective_compute(
                "AllGather",
                mybir.AluOpType.bypass,
                replica_groups=[list(range(R))],
                ins=[shard1.opt()],
                outs=[table1.opt()],
            )

            if phases == "A":
                dbg = wp.tile([128, 64], f32, tag="dbg")
                nc.sync.dma_start(out=dbg[:], in_=table1[0:128, 0:64])
                nc.sync.dma_start(out=out_d[0:128, :], in_=dbg[:])

            # ---------------- phase B: layer-1 attention + aggregation
            col = [0]  # running idx column offset
            qrr = [0]  # SWDGE queue round-robin

            def gather_pair(table, EL, t0, J, Dl, Dh):
                outs = []
                for D, base0, base1 in ((Dl, 0, HALF), (Dh, HALF, R * NPAD)):
                    NI = 128 * J * D
                    g = gp.tile([128, J * D * EL], f32, tag="g")
                    nc.gpsimd.dma_gather(
                        g[:].rearrange("p (c r) -> p c r", r=EL),
                        table[base0:base1, :],
                        idxall[:, col[0] : col[0] + NI // 16],
                        NI,
                        NI,
                        EL,
                        single_packet=False,
                        queue_num=qrr[0] % 4,
                    )
                    qrr[0] += 1
                    col[0] += NI // 16
                    outs.append(g)
                return outs

            def attention_tile(g, EL, CCH, NH, D, J, j, ad_ap):
                """Per 128-node subtile: e -> exp, partial den [128,NH] and
                scr reduce -> raw [128, NH*CCH]. Returns (den, raw)."""
                gv = g[:].rearrange("p (k J r) -> p r k J", J=J, r=EL)
                as_v = gv[:, CCH * NH : CCH * NH + NH, :, j]  # [p, NH, D]
                e = wp.tile([128, NH * D], f32, tag="e")
                if NH > 1:
                    nc.vector.tensor_tensor(
                        out=e[:].rearrange("p (h k) -> p h k", h=NH),
                        in0=as_v,
                        in1=ad_ap.unsqueeze(2).to_broadcast([128, NH, D]),
                        op=OP.add,
                    )
                else:
                    nc.scalar.activation(e[:], as_v[:, 0, :], AF.Identity, bias=ad_ap)
                # leaky_relu(x) = max(x, 0.2 x); scale on ACT, max on DVE
                esc = wp.tile([128, NH * D], f32, tag="esc")
                nc.scalar.mul(esc[:], e[:], NEG_SLOPE)
                nc.vector.tensor_tensor(e[:], e[:], esc[:], op=OP.max)
                ex = wp.tile([128, NH * D], f32, tag="ex")
                nc.scalar.activation(ex[:], e[:], AF.Exp)
                den = wp.tile([128, NH], f32, tag="den")
                nc.vector.reduce_sum(
                    out=den[:], in_=ex[:].rearrange("p (h k) -> p h k", h=NH), axis=AX
                )
                # payload view iterated (h, k, c): c innermost is contiguous
                h_v = g[:].rearrange("p (k J r) -> p k J r", J=J, r=EL)[
                    :, :, j, 0 : CCH * NH
                ].rearrange("p k (h c) -> p h k c", h=NH)  # [p, NH, D, CCH]
                scr = wp.tile([128, NH * CCH * D], f32, tag="scr")
                nc.vector.tensor_tensor(
                    out=scr[:].rearrange("p (h k c) -> p h k c", h=NH, k=D),
                    in0=h_v,
                    in1=ex[:]
                    .rearrange("p (h k) -> p h k", h=NH)
                    .unsqueeze(3)
                    .to_broadcast([128, NH, D, CCH]),
                    op=OP.mult,
                )
                raw = wp.tile([128, NH * CCH], f32, tag="raw")
                nc.vector.reduce_sum(
                    out=raw[:],
                    in_=scr[:].rearrange("p (h k c) -> p h c k", h=NH, k=D),
                    axis=AX,
                )
                return den, raw

            for t0, J, Dl, Dh in supers if "B" in phases else []:
                glo, ghi = gather_pair(table1, EL1, t0, J, Dl, Dh)
                if "G" in phases:  # gathers only: consume via a dummy copy
                    dbg2 = wp.tile([128, 64], f32, tag="dbg2")
                    nc.vector.tensor_copy(out=dbg2[:], in_=glo[:, 0:64])
                    nc.vector.tensor_copy(out=dbg2[:], in_=ghi[:, 0:64])
                    nc.sync.dma_start(
                        out=out_d[t0 * 128 : (t0 + 1) * 128, :], in_=dbg2[:]
                    )
                    continue
                for j in range(J):
                    t = t0 + j
                    ad = adbuf1[:, 4 * t : 4 * t + 4]
                    den_l, raw_l = attention_tile(glo, EL1, HIDDEN, HEADS, Dl, J, j, ad)
                    den_h, raw_h = attention_tile(ghi, EL1, HIDDEN, HEADS, Dh, J, j, ad)
                    den = wp.tile([128, 4], f32, tag="dent")
                    nc.vector.tensor_tensor(den[:], den_l[:], den_h[:], op=OP.add)
                    nc.vector.tensor_scalar_add(den[:], den[:], 1e-16)
                    rden = wp.tile([128, 4], f32, tag="rden")
                    nc.vector.reciprocal(rden[:], den[:])
                    raw = wp.tile([128, 128], f32, tag="rawt")
                    nc.vector.tensor_tensor(raw[:], raw_l[:], raw_h[:], op=OP.add)
                    out1 = wp.tile([128, 128], f32, tag="out1")
                    nc.vector.tensor_tensor(
                        out=out1[:].rearrange("p (h c) -> p h c", h=4),
                        in0=raw[:].rearrange("p (h c) -> p h c", h=4),
                        in1=rden[:].unsqueeze(2).to_broadcast([128, 4, HIDDEN]),
                        op=OP.mult,
                    )
                    if "R" in phases:  # stop after aggregation
                        nc.sync.dma_start(
                            out=out_d[t * 128 : (t + 1) * 128, :], in_=out1[:, 0:64]
                        )
                        continue
                    # transpose -> [c, n], ELU(z + b1), then @ [W2|Ws2|Wd2]
                    psT = pp.tile([128, 128], f32, tag="psT")
                    nc.tensor.transpose(psT[:], out1[:], ident[:])
                    zt = wp.tile([128, 128], f32, tag="zt")
                    nc.scalar.activation(zt[:], psT[:], AF.Identity, bias=b1c[:, :1])
                    # ELU on ACT: exp(min(z,0)) = exp(-relu(-z)); relu(z) on ACT too
                    mt = wp.tile([128, 128], f32, tag="mt")
                    nc.scalar.activation(mt[:], zt[:], AF.Relu, scale=-1.0)
                    emt = wp.tile([128, 128], f32, tag="emt")
                    nc.scalar.activation(emt[:], mt[:], AF.Exp, scale=-1.0)
                    rt = wp.tile([128, 128], f32, tag="rt")
                    nc.scalar.activation(rt[:], zt[:], AF.Relu)
                    nc.vector.tensor_tensor(rt[:], rt[:], emt[:], op=OP.add)
                    elut = wp.tile([128, 128], f32, tag="elut")
                    nc.vector.tensor_scalar_add(elut[:], rt[:], -1.0)
                    ps2 = pp.tile([128, 66], f32, tag="ps2")
                    nc.tensor.matmul(
                        ps2[:], lhsT=elut[:], rhs=wcat2[:], start=True, stop=True
                    )
                    h2e = wp.tile([128, 66], f32, tag="h2e")
                    nc.scalar.copy(out=h2e[:], in_=ps2[:])
                    nc.vector.tensor_copy(out=adbuf2[:, t : t + 1], in_=h2e[:, 65:66])
                    nc.sync.dma_start(
                        out=shard2[t * 128 : (t + 1) * 128, 0:65], in_=h2e[:, 0:65]
                    )
                    nc.sync.dma_start(
                        out=shard2[t * 128 : (t + 1) * 128, 65:EL2],
                        in_=zpad[:, 0 : EL2 - 65],
                    )
                    if "C" not in phases:
                        nc.sync.dma_start(
                            out=out_d[t * 128 : (t + 1) * 128, :], in_=out1[:, 0:64]
                        )

            if "C" in phases:
                nc.sync.dma_start(
                    out=shard2[NPC:NPAD, 64:65], in_=neg[0 : NPAD - NPC, 0:1]
                )
                nc.gpsimd.collective_compute(
                    "AllGather",
                    mybir.AluOpType.bypass,
                    replica_groups=[list(range(R))],
                    ins=[shard2.opt()],
                    outs=[table2.opt()],
                )

            # ---------------- phase C: layer-2 attention + aggregation
            col2 = col[0]
            col[0] = 0
            for t0, J, Dl, Dh in supers if "C" in phases else []:
                glo, ghi = gather_pair(table2, EL2, t0, J, Dl, Dh)
                for j in range(J):
                    t = t0 + j
                    ad = adbuf2[:, t : t + 1]
                    den_l, raw_l = attention_tile(glo, EL2, OUT_CH, 1, Dl, J, j, ad)
                    den_h, raw_h = attention_tile(ghi, EL2, OUT_CH, 1, Dh, J, j, ad)
                    den = wp.tile([128, 1], f32, tag="dent2")
                    nc.vector.tensor_tensor(den[:], den_l[:], den_h[:], op=OP.add)
                    nc.vector.tensor_scalar_add(den[:], den[:], 1e-16)
                    rden = wp.tile([128, 1], f32, tag="rden2")
                    nc.vector.reciprocal(rden[:], den[:])
                    raw = wp.tile([128, 64], f32, tag="rawt2")
                    nc.vector.tensor_tensor(raw[:], raw_l[:], raw_h[:], op=OP.add)
                    out2 = wp.tile([128, 64], f32, tag="out2")
                    nc.vector.tensor_tensor(
                        out=out2[:],
                        in0=raw[:],
                        in1=rden[:].to_broadcast([128, 64]),
                        op=OP.mult,
                    )
                    nc.vector.tensor_tensor(out2[:], out2[:], b2b[:], op=OP.add)
                    nc.sync.dma_start(
                        out=out_d[t * 128 : (t + 1) * 128, :], in_=out2[:]
                    )
            assert "C" not in phases or col[0] == col2

    nc.finalize()
    return nc


# ---------------------------------------------------------------- entry point
_cache = {}


def kernel(x, edge_index, W1, att_src1, att_dst1, b1, W2, att_src2, att_dst2, b2):
    from concourse.bass_utils import run_bass_kernel_spmd

    x = np.asarray(x, dtype=np.float32)
    edge_index = np.asarray(edge_index, dtype=np.int64)
    W1 = np.asarray(W1, dtype=np.float32)
    W2 = np.asarray(W2, dtype=np.float32)
    att_src1 = np.asarray(att_src1, dtype=np.float32)
    att_dst1 = np.asarray(att_dst1, dtype=np.float32)
    att_src2 = np.asarray(att_src2, dtype=np.float32)
    att_dst2 = np.asarray(att_dst2, dtype=np.float32)
    b1 = np.asarray(b1, dtype=np.float32)
    b2 = np.asarray(b2, dtype=np.float32)

    import os

    phases = os.environ.get("KERNEL_PHASES", "ABC")
    key = (hash(edge_index.tobytes()), phases)  # cache key for repeated calls
    if "plan" not in _cache or _cache.get("key") != key:
        _cache["plan"] = _build_plan(edge_index)
        _cache["nc"] = _build_bass(_cache["plan"], phases)
        _cache["key"] = key
    plan = _cache["plan"]
    nc = _cache["nc"]

    # weight packing: as = x @ (W1 . att_src) etc.
    W1r = W1.reshape(IN_CH, HEADS, HIDDEN)
    Ws1 = np.einsum("khc,hc->kh", W1r, att_src1)  # [128, 4]
    Wd1 = np.einsum("khc,hc->kh", W1r, att_dst1)
    wcat1 = np.concatenate([W1, Ws1, Wd1], axis=1).astype(np.float32)  # [128,136]
    Ws2 = W2 @ att_src2[0]  # [128]
    Wd2 = W2 @ att_dst2[0]
    wcat2 = np.concatenate([W2, Ws2[:, None], Wd2[:, None]], axis=1).astype(np.float32)

    in_maps = []
    for c in range(R):
        xp = np.zeros((NPAD, IN_CH), dtype=np.float32)
        xp[:NPC] = x[plan["perms"][c]]
        in_maps.append(
            {
                "xT": np.ascontiguousarray(xp.T),
                "gidx": plan["gidx"][c],
                "wcat1": wcat1,
                "wcat2": wcat2,
                "b1c": b1.reshape(128, 1).astype(np.float32),
                "b2r": b2.reshape(1, 64).astype(np.float32),
            }
        )

    res = run_bass_kernel_spmd(nc, in_maps, core_ids=list(range(R)))
    _cache["last_res"] = res
    out = np.empty((N, OUT_CH), dtype=np.float32)
    for c in range(R):
        out[plan["perms"][c]] = res.results[c]["out"][:NPC]
    return out


# revision 32
# speedup vs baseline: 1.7375x; 1.7375x over previous
"""2-layer GAT on 8 Trainium2 NeuronCores.

Strategy: dst-shard nodes across cores (6250 each, padded to 6272). Per layer:
each core computes node features h = x @ [W | W@att_src | W@att_dst] for its
shard, AllGathers the packed per-node rows into a replicated DRAM table, then
processes its own dst nodes in CSR tiles (128 nodes on partitions, slots along
free dim): dma_gather pulls [h | alpha_src] rows for every incoming edge,
softmax and the alpha-weighted payload sum are dense strided vector ops.
int16 gather indices limit one gather to 32768 table rows, so each tile does
two gathers (table halves lo/hi) and sums the partial numerators/denominators.
"""

import numpy as np

N = 50000
E = 800000
R = 8
NPC = N // R  # 6250 owned nodes per core
TPC = 49  # tiles of 128 nodes
NPAD = TPC * 128  # 6272 rows per shard
HALF = 4 * NPAD  # 25088 table rows per half
IN_CH = 128
HIDDEN = 32
HEADS = 4
OUT_CH = 64
NEG_SLOPE = 0.2
EL1 = 192  # L1 table row: [h1(128) | as1(4) | pad] f32, 768B
EL2 = 128  # L2 table row: [h2(64) | as2(1) | pad] f32, 512B
SENT = 6250  # sentinel row (first pad row of core 0 / core 4) in each half
CAP = 34  # supertile packing: J*(D_lo+D_hi) <= CAP


# ---------------------------------------------------------------- host planner
def _build_plan(edge_index):
    src = np.concatenate([edge_index[0], np.arange(N, dtype=np.int64)]).astype(np.int64)
    dst = np.concatenate([edge_index[1], np.arange(N, dtype=np.int64)]).astype(np.int64)
    lo = src < (N // 2)  # src owned by cores 0-3 -> table half 0

    # degree per (dst, half)
    d_lo = np.bincount(dst[lo], minlength=N)
    d_hi = np.bincount(dst[~lo], minlength=N)

    # per-core permutation: lex sort desc by (d_lo, d_hi); pads (deg 0) at end
    perms = []  # perms[c] = array of orig node ids, len NPC, permuted order
    pos = np.empty(N, dtype=np.int64)  # node id -> permuted global row
    for c in range(R):
        ids = np.arange(c * NPC, (c + 1) * NPC)
        key = np.maximum(d_lo[ids], d_hi[ids]) * 1000 + d_lo[ids] + d_hi[ids]
        order = np.argsort(-key, kind="stable")
        p = ids[order]
        perms.append(p)
        pos[p] = c * NPAD + np.arange(NPC)

    # shared per-tile max degrees (padded rows have degree 0)
    dlo_t = np.zeros(TPC, dtype=np.int64)
    dhi_t = np.zeros(TPC, dtype=np.int64)
    for c in range(R):
        dl = d_lo[perms[c]]
        dh = d_hi[perms[c]]
        dl = np.concatenate([dl, np.zeros(NPAD - NPC, dtype=np.int64)])
        dh = np.concatenate([dh, np.zeros(NPAD - NPC, dtype=np.int64)])
        dlo_t = np.maximum(dlo_t, dl.reshape(TPC, 128).max(axis=1))
        dhi_t = np.maximum(dhi_t, dh.reshape(TPC, 128).max(axis=1))
    dlo_t = np.maximum(dlo_t, 1)
    dhi_t = np.maximum(dhi_t, 1)

    # supertiles: group J consecutive tiles, J in {4,2,1}
    supers = []  # (t0, J, Dl, Dh)
    t = 0
    while t < TPC:
        for J in (4, 2, 1):
            if t + J <= TPC:
                Dl = int(dlo_t[t : t + J].max())
                Dh = int(dhi_t[t : t + J].max())
                if J * (Dl + Dh) <= CAP or J == 1:
                    break
        supers.append((t, J, Dl, Dh))
        t += J

    slots = sum(128 * J * (Dl + Dh) for (_, J, Dl, Dh) in supers)
    real = E + N
    plan = {
        "supers": supers,
        "perms": perms,
        "pos": pos,
        "inflation": slots * R / real,
    }

    # per-core slot tables: for owned dst nodes, list of src-pos per half
    # cumcount of edges within (dst, half)
    gidx_cores = []
    W_total = sum(8 * 128 * J * (Dl + Dh) // 128 for (_, J, Dl, Dh) in supers)
    # columns of wrapped idx per gather = NI/16 = 128*J*D/16 = 8*J*D
    for c in range(R):
        own = (dst >= c * NPC) & (dst < (c + 1) * NPC)
        s_own = src[own]
        d_own = dst[own]
        half = (~(s_own < (N // 2))).astype(np.int64)  # 0 lo, 1 hi
        lpos = pos[d_own] - c * NPAD  # local permuted pos of dst, [0, NPC)
        key = lpos * 2 + half
        order = np.argsort(key, kind="stable")
        key_s = key[order]
        sp = pos[s_own][order]  # permuted global row of src
        # rank within group
        first = np.searchsorted(key_s, key_s)  # index of first occurrence
        rank = np.arange(len(key_s)) - first

        # slot arrays per tile: build dense [NPAD, Dl/Dh] with sentinel fill
        cols = []
        for t0, J, Dl, Dh in supers:
            n0 = t0 * 128
            n1 = (t0 + J) * 128
            for h, D, base, sent in ((0, Dl, 0, SENT), (1, Dh, HALF, HALF + SENT)):
                tab = np.full((n1 - n0, D), sent - base, dtype=np.int64)
                sel = (key_s % 2 == h) & (key_s // 2 >= n0) & (key_s // 2 < n1)
                rr = rank[sel]
                keep = rr < D
                tab[key_s[sel][keep] // 2 - n0, rr[keep]] = sp[sel][keep] - base
                assert keep.all(), "rank exceeded tile max degree"
                # flat[i], i = c*128 + p, c = k*J + j, node = j*128 + p
                # tab is [S, D] with S = J*128 node-pos-major
                S = n1 - n0
                flat = np.empty(S * D, dtype=np.int16)
                nodes = np.arange(S)
                j = nodes // 128
                p = nodes % 128
                for k in range(D):
                    flat[(k * J + j) * 128 + p] = tab[nodes, k]
                wrapped = flat.reshape(-1, 16)  # [NI/16, 16], flat[s*16 + q]
                w = np.empty((128, S * D // 16), dtype=np.int16)
                for q in range(8):
                    w[q * 16 : (q + 1) * 16, :] = wrapped.T
                cols.append(w)
        gidx_cores.append(np.concatenate(cols, axis=1))
    plan["gidx"] = gidx_cores
    plan["W"] = gidx_cores[0].shape[1]
    assert all(g.shape[1] == plan["W"] for g in gidx_cores)
    return plan


# ---------------------------------------------------------------- bass kernel
def _build_bass(plan, phases="ABC"):
    import concourse.bacc as bacc
    import concourse.bass as bass
    import concourse.mybir as mybir
    import concourse.tile as tile
    from concourse.masks import make_identity

    f32 = mybir.dt.float32
    i16 = mybir.dt.int16
    AX = mybir.AxisListType.X
    OP = mybir.AluOpType
    AF = mybir.ActivationFunctionType

    supers = plan["supers"]
    W = plan["W"]

    nc = bacc.Bacc(
        "TRN2",
        target_bir_lowering=False,
        debug=False,
        num_devices=R,
        num_swdge_queues=4,
        dynamic_dma_scratch_size=32768,
    )
    xT_in = nc.dram_tensor("xT", [128, NPAD], f32, kind="ExternalInput")
    gidx_in = nc.dram_tensor("gidx", [128, W], i16, kind="ExternalInput")
    wcat1_in = nc.dram_tensor("wcat1", [128, 136], f32, kind="ExternalInput")
    wcat2_in = nc.dram_tensor("wcat2", [128, 66], f32, kind="ExternalInput")
    b1_in = nc.dram_tensor("b1c", [128, 1], f32, kind="ExternalInput")
    b2_in = nc.dram_tensor("b2r", [1, 64], f32, kind="ExternalInput")
    out_d = nc.dram_tensor("out", [NPAD, 64], f32, kind="ExternalOutput")

    with tile.TileContext(nc) as tc:
        with (
            tc.tile_pool(name="const", bufs=1) as cp,
            tc.tile_pool(name="work", bufs=3) as wp,
            tc.tile_pool(name="gath", bufs=4) as gp,
            tc.tile_pool(name="psum", bufs=2, space="PSUM") as pp,
            tc.tile_pool(name="dram", bufs=1, space="DRAM") as dp,
        ):
            shard1 = dp.tile([NPAD, EL1], f32)
            table1 = dp.tile([R * NPAD, EL1], f32, addr_space="Shared")
            shard2 = dp.tile([NPAD, EL2], f32)
            table2 = dp.tile([R * NPAD, EL2], f32, addr_space="Shared")

            wcat1 = cp.tile([128, 136], f32)
            nc.sync.dma_start(out=wcat1[:], in_=wcat1_in[:])
            wcat2 = cp.tile([128, 66], f32)
            nc.sync.dma_start(out=wcat2[:], in_=wcat2_in[:])
            b1c = cp.tile([128, 1], f32)
            nc.sync.dma_start(out=b1c[:], in_=b1_in[:])
            b2p = cp.tile([1, 64], f32)
            nc.sync.dma_start(out=b2p[:1, :], in_=b2_in[:])
            b2b = cp.tile([128, 64], f32)
            nc.gpsimd.partition_broadcast(b2b[:], b2p[:1, :])
            ident = cp.tile([128, 128], f32)
            make_identity(nc, ident[:])
            idxall = cp.tile([128, W], i16)
            nc.sync.dma_start(out=idxall[:], in_=gidx_in[:])
            adbuf1 = cp.tile([128, 4 * TPC], f32)
            adbuf2 = cp.tile([128, TPC], f32)
            neg = cp.tile([128, 4], f32)
            nc.gpsimd.memset(neg[:], -1e30)
            zpad = cp.tile([128, 64], f32)
            nc.gpsimd.memset(zpad[:], 0.0)

            # ---------------- phase A: h1ext = x @ [W1|Ws1|Wd1] per owned tile
            for t in range(TPC):
                xt = wp.tile([128, 128], f32, tag="xt")
                nc.sync.dma_start(out=xt[:], in_=xT_in[:, t * 128 : (t + 1) * 128])
                psA = pp.tile([128, 136], f32, tag="psA")
                nc.tensor.matmul(psA[:], lhsT=xt[:], rhs=wcat1[:], start=True, stop=True)
                hext = wp.tile([128, 136], f32, tag="hext")
                nc.scalar.copy(out=hext[:], in_=psA[:])
                nc.vector.tensor_copy(
                    out=adbuf1[:, 4 * t : 4 * t + 4], in_=hext[:, 132:136]
                )
                nc.sync.dma_start(
                    out=shard1[t * 128 : (t + 1) * 128, 0:132], in_=hext[:, 0:132]
                )
                nc.sync.dma_start(
                    out=shard1[t * 128 : (t + 1) * 128, 132:EL1],
                    in_=zpad[:, 0 : EL1 - 132],
                )
            # pad rows: alpha_src = -1e30 so padded slots vanish in the softmax
            nc.sync.dma_start(
                out=shard1[NPC:NPAD, 128:132], in_=neg[0 : NPAD - NPC, 0:4]
            )

            nc.gpsimd.collective_compute(
                "AllGather",
                mybir.AluOpType.bypass,
                replica_groups=[list(range(R))],
                ins=[shard1.opt()],
                outs=[table1.opt()],
            )

            if phases == "A":
                dbg = wp.tile([128, 64], f32, tag="dbg")
                nc.sync.dma_start(out=dbg[:], in_=table1[0:128, 0:64])
                nc.sync.dma_start(out=out_d[0:128, :], in_=dbg[:])

            # ---------------- phase B: layer-1 attention + aggregation
            col = [0]  # running idx column offset
            qrr = [0]  # SWDGE queue round-robin

            def gather_pair(table, EL, t0, J, Dl, Dh):
                outs = []
                for D, base0, base1 in ((Dl, 0, HALF), (Dh, HALF, R * NPAD)):
                    NI = 128 * J * D
                    g = gp.tile([128, J * D * EL], f32, tag="g")
                    nc.gpsimd.dma_gather(
                        g[:].rearrange("p (c r) -> p c r", r=EL),
                        table[base0:base1, :],
                        idxall[:, col[0] : col[0] + NI // 16],
                        NI,
                        NI,
                        EL,
                        single_packet=False,
                        queue_num=qrr[0] % 4,
                    )
                    qrr[0] += 1
                    col[0] += NI // 16
                    outs.append(g)
                return outs

            def attention_tile(g, EL, CCH, NH, D, J, j, ad_ap):
                """Per 128-node subtile: e -> exp, partial den [128,NH] and
                scr reduce -> raw [128, NH*CCH]. Returns (den, raw)."""
                gv = g[:].rearrange("p (k J r) -> p r k J", J=J, r=EL)
                as_v = gv[:, CCH * NH : CCH * NH + NH, :, j]  # [p, NH, D]
                e = wp.tile([128, NH * D], f32, tag="e")
                if NH > 1:
                    nc.vector.tensor_tensor(
                        out=e[:].rearrange("p (h k) -> p h k", h=NH),
                        in0=as_v,
                        in1=ad_ap.unsqueeze(2).to_broadcast([128, NH, D]),
                        op=OP.add,
                    )
                else:
                    nc.scalar.activation(e[:], as_v[:, 0, :], AF.Identity, bias=ad_ap)
                # leaky_relu(x) = max(x, 0.2 x); scale on ACT, max on DVE
                esc = wp.tile([128, NH * D], f32, tag="esc")
                nc.scalar.mul(esc[:], e[:], NEG_SLOPE)
                nc.vector.tensor_tensor(e[:], e[:], esc[:], op=OP.max)
                ex = wp.tile([128, NH * D], f32, tag="ex")
                nc.scalar.activation(ex[:], e[:], AF.Exp)
                den = wp.tile([128, NH], f32, tag="den")
                nc.vector.reduce_sum(
                    out=den[:], in_=ex[:].rearrange("p (h k) -> p h k", h=NH), axis=AX
                )
                # payload view iterated (h, k, c): c innermost is contiguous
                h_v = g[:].rearrange("p (k J r) -> p k J r", J=J, r=EL)[
                    :, :, j, 0 : CCH * NH
                ].rearrange("p k (h c) -> p h k c", h=NH)  # [p, NH, D, CCH]
                scr = wp.tile([128, NH * CCH * D], f32, tag="scr")
                nc.vector.tensor_tensor(
                    out=scr[:].rearrange("p (h k c) -> p h k c", h=NH, k=D),
                    in0=h_v,
                    in1=ex[:]
                    .rearrange("p (h k) -> p h k", h=NH)
                    .unsqueeze(3)
                    .to_broadcast([128, NH, D, CCH]),
                    op=OP.mult,
                )
                raw = wp.tile([128, NH * CCH], f32, tag="raw")
                nc.vector.reduce_sum(
                    out=raw[:],
                    in_=scr[:].rearrange("p (h k c) -> p h c k", h=NH, k=D),
                    axis=AX,
                )
                return den, raw

            for t0, J, Dl, Dh in supers if "B" in phases else []:
                glo, ghi = gather_pair(table1, EL1, t0, J, Dl, Dh)
                if "G" in phases:  # gathers only: consume via a dummy copy
                    dbg2 = wp.tile([128, 64], f32, tag="dbg2")
                    nc.vector.tensor_copy(out=dbg2[:], in_=glo[:, 0:64])
                    nc.vector.tensor_copy(out=dbg2[:], in_=ghi[:, 0:64])
                    nc.sync.dma_start(
                        out=out_d[t0 * 128 : (t0 + 1) * 128, :], in_=dbg2[:]
                    )
                    continue
                for j in range(J):
                    t = t0 + j
                    ad = adbuf1[:, 4 * t : 4 * t + 4]
                    den_l, raw_l = attention_tile(glo, EL1, HIDDEN, HEADS, Dl, J, j, ad)
                    den_h, raw_h = attention_tile(ghi, EL1, HIDDEN, HEADS, Dh, J, j, ad)
                    den = wp.tile([128, 4], f32, tag="dent")
                    nc.vector.tensor_tensor(den[:], den_l[:], den_h[:], op=OP.add)
                    nc.vector.tensor_scalar_add(den[:], den[:], 1e-16)
                    rden = wp.tile([128, 4], f32, tag="rden")
                    nc.vector.reciprocal(rden[:], den[:])
                    raw = wp.tile([128, 128], f32, tag="rawt")
                    nc.vector.tensor_tensor(raw[:], raw_l[:], raw_h[:], op=OP.add)
                    out1 = wp.tile([128, 128], f32, tag="out1")
                    nc.vector.tensor_tensor(
                        out=out1[:].rearrange("p (h c) -> p h c", h=4),
                        in0=raw[:].rearrange("p (h c) -> p h c", h=4),
                        in1=rden[:].unsqueeze(2).to_broadcast([128, 4, HIDDEN]),
                        op=OP.mult,
                    )
                    if "R" in phases:  # stop after aggregation
                        nc.sync.dma_start(
                            out=out_d[t * 128 : (t + 1) * 128, :], in_=out1[:, 0:64]
                        )
                        continue
                    # transpose -> [c, n], ELU(z + b1), then @ [W2|Ws2|Wd2]
                    psT = pp.tile([128, 128], f32, tag="psT")
                    nc.tensor.transpose(psT[:], out1[:], ident[:])
                    zt = wp.tile([128, 128], f32, tag="zt")
                    nc.scalar.activation(zt[:], psT[:], AF.Identity, bias=b1c[:, :1])
                    # ELU on ACT: exp(min(z,0)) = exp(-relu(-z)); relu(z) on ACT too
                    mt = wp.tile([128, 128], f32, tag="mt")
                    nc.scalar.activation(mt[:], zt[:], AF.Relu, scale=-1.0)
                    emt = wp.tile([128, 128], f32, tag="emt")
                    nc.scalar.activation(emt[:], mt[:], AF.Exp, scale=-1.0)
                    rt = wp.tile([128, 128], f32, tag="rt")
                    nc.scalar.activation(rt[:], zt[:], AF.Relu)
                    nc.vector.tensor_tensor(rt[:], rt[:], emt[:], op=OP.add)
                    elut = wp.tile([128, 128], f32, tag="elut")
                    nc.vector.tensor_scalar_add(elut[:], rt[:], -1.0)
                    ps2 = pp.tile([128, 66], f32, tag="ps2")
                    nc.tensor.matmul(
                        ps2[:], lhsT=elut[:], rhs=wcat2[:], start=True, stop=True
                    )
                    h2e = wp.tile([128, 66], f32, tag="h2e")
                    nc.scalar.copy(out=h2e[:], in_=ps2[:])
                    nc.vector.tensor_copy(out=adbuf2[:, t : t + 1], in_=h2e[:, 65:66])
                    nc.sync.dma_start(
                        out=shard2[t * 128 : (t + 1) * 128, 0:65], in_=h2e[:, 0:65]
                    )
                    nc.sync.dma_start(
                        out=shard2[t * 128 : (t + 1) * 128, 65:EL2],
                        in_=zpad[:, 0 : EL2 - 65],
                    )
                    if "C" not in phases:
                        nc.sync.dma_start(
                            out=out_d[t * 128 : (t + 1) * 128, :], in_=out1[:, 0:64]
                        )

            if "C" in phases:
                nc.sync.dma_start(
                    out=shard2[NPC:NPAD, 64:65], in_=neg[0 : NPAD - NPC, 0:1]
                )
                nc.gpsimd.collective_compute(
                    "AllGather",
                    mybir.AluOpType.bypass,
                    replica_groups=[list(range(R))],
                    ins=[shard2.opt()],
                    outs=[table2.opt()],
                )

            # ---------------- phase C: layer-2 attention + aggregation
            col2 = col[0]
            col[0] = 0
            for t0, J, Dl, Dh in supers if "C" in phases else []:
                glo, ghi = gather_pair(table2, EL2, t0, J, Dl, Dh)
                for j in range(J):
                    t = t0 + j
                    ad = adbuf2[:, t : t + 1]
                    den_l, raw_l = attention_tile(glo, EL2, OUT_CH, 1, Dl, J, j, ad)
                    den_h, raw_h = attention_tile(ghi, EL2, OUT_CH, 1, Dh, J, j, ad)
                    den = wp.tile([128, 1], f32, tag="dent2")
                    nc.vector.tensor_tensor(den[:], den_l[:], den_h[:], op=OP.add)
                    nc.vector.tensor_scalar_add(den[:], den[:], 1e-16)
                    rden = wp.tile([128, 1], f32, tag="rden2")
                    nc.vector.reciprocal(rden[:], den[:])
                    raw = wp.tile([128, 64], f32, tag="rawt2")
                    nc.vector.tensor_tensor(raw[:], raw_l[:], raw_h[:], op=OP.add)
                    out2 = wp.tile([128, 64], f32, tag="out2")
                    nc.vector.tensor_tensor(
                        out=out2[:],
                        in0=raw[:],
                        in1=rden[:].to_broadcast([128, 64]),
                        op=OP.mult,
                    )
                    nc.vector.tensor_tensor(out2[:], out2[:], b2b[:], op=OP.add)
                    nc.sync.dma_start(
                        out=out_d[t * 128 : (t + 1) * 128, :], in_=out2[:]
                    )
            assert "C" not in phases or col[0] == col2

    nc.finalize()
    return nc


# ---------------------------------------------------------------- entry point
_cache = {}


def kernel(x, edge_index, W1, att_src1, att_dst1, b1, W2, att_src2, att_dst2, b2):
    from concourse.bass_utils import run_bass_kernel_spmd

    x = np.asarray(x, dtype=np.float32)
    edge_index = np.asarray(edge_index, dtype=np.int64)
    W1 = np.asarray(W1, dtype=np.float32)
    W2 = np.asarray(W2, dtype=np.float32)
    att_src1 = np.asarray(att_src1, dtype=np.float32)
    att_dst1 = np.asarray(att_dst1, dtype=np.float32)
    att_src2 = np.asarray(att_src2, dtype=np.float32)
    att_dst2 = np.asarray(att_dst2, dtype=np.float32)
    b1 = np.asarray(b1, dtype=np.float32)
    b2 = np.asarray(b2, dtype=np.float32)

    import os

    phases = os.environ.get("KERNEL_PHASES", "ABC")
    key = (hash(edge_index.tobytes()), phases)  # cache key for repeated calls
    if "plan" not in _cache or _cache.get("key") != key:
        _cache["plan"] = _build_plan(edge_index)
        _cache["nc"] = _build_bass(_cache["plan"], phases)
        _cache["key"] = key
    plan = _cache["plan"]
    nc = _cache["nc"]

    # weight packing: as = x @ (W1 . att_src) etc.
    W1r = W1.reshape(IN_CH, HEADS, HIDDEN)
    Ws1 = np.einsum("khc,hc->kh", W1r, att_src1)  # [128, 4]
    Wd1 = np.einsum("khc,hc->kh", W1r, att_dst1)
    wcat1 = np.concatenate([W1, Ws1, Wd1], axis=1).astype(np.float32)  # [128,136]
    Ws2 = W2 @ att_src2[0]  # [128]
    Wd2 = W2 @ att_dst2[0]
    wcat2 = np.concatenate([W2, Ws2[:, None], Wd2[:, None]], axis=1).astype(np.float32)

    in_maps = []
    for c in range(R):
        xp = np.zeros((NPAD, IN_CH), dtype=np.float32)
        xp[:NPC] = x[plan["perms"][c]]
        in_maps.append(
            {
                "xT": np.ascontiguousarray(xp.T),
                "gidx": plan["gidx"][c],
                "wcat1": wcat1,
                "wcat2": wcat2,
                "b1c": b1.reshape(128, 1).astype(np.float32),
                "b2r": b2.reshape(1, 64).astype(np.float32),
            }
        )

    res = run_bass_kernel_spmd(nc, in_maps, core_ids=list(range(R)))
    _cache["last_res"] = res
    out = np.empty((N, OUT_CH), dtype=np.float32)
    for c in range(R):
        out[plan["perms"][c]] = res.results[c]["out"][:NPC]
    return out
